# revision 1
# baseline (speedup 1.0000x reference)
"""Multi-head attention (RoPE, causal) on 8 TRN2 NeuronCores.

Sharding: core c -> batch b = c//2, head-group g = c%2 (8 of 16 heads).

v3: B1 (q/k projections + rope) is interleaved with attention per
p-group so ACT-bound attention overlaps PE-bound projections.  One
shared PSUM pool: pab 2 banks + pS 4 banks + pO 2 banks = 8.
"""

import numpy as np
from contextlib import ExitStack

import concourse.bacc as bacc
import concourse.bass as bass
import concourse.mybir as mybir
import concourse.tile as tile
from concourse.bass_utils import run_bass_kernel_spmd
from concourse.masks import make_identity, make_upper_triangular

F32 = mybir.dt.float32
F32R = mybir.dt.float32r
AF = mybir.ActivationFunctionType

D = 1024
S = 2048
NH = 16
DK = 64
HPC = 8          # heads per core
HD = HPC * DK    # 512
NCORES = 8
THETA = 10000.0

NS = S // 128    # 16 s-tiles
NC_ = 4          # s-chunks of 512
NK = D // 128    # 8 k-tiles
HF = 1024        # attention sq-half width

_CACHE = {}


def _copy(nc, use_scalar, out, in_):
    if use_scalar:
        nc.scalar.copy(out, in_)
    else:
        nc.vector.tensor_copy(out, in_)


def _emit_xT(nc, tc, psp, XT, xtall, ident):
    """Load host-pre-transposed x directly into xtall (4 chunked DMAs)."""
    for g in range(4):
        eng = nc.sync if g % 2 == 0 else nc.scalar
        eng.dma_start(
            out=xtall[:, 2 * g:2 * g + 2, :],
            in_=XT[256 * g:256 * (g + 1), :].rearrange(
                "(k r) s -> r k s", r=128))


def _emit_v(nc, tc, psp, WV, xtall, vpall, ones8):
    """V in natural layout + interleaved ones columns -> vpall."""
    with tc.tile_pool(name="wvp", bufs=9) as wvp:
        wv = []
        for k in range(NK):
            w = wvp.tile([128, HD], F32R, tag="wv")
            nc.sync.dma_start(out=w, in_=WV[k * 128:(k + 1) * 128, :])
            wv.append(w)
        for j in range(NS):
            psv = psp.tile([128, 512], F32, tag="pab", name="psv")
            for k in range(NK):
                nc.tensor.matmul(
                    psv, xtall[:, k, j * 128:(j + 1) * 128], wv[k],
                    start=(k == 0), stop=(k == NK - 1))
            vslice = vpall[:, j, :].rearrange("p (h e) -> p h e", e=65)
            _copy(nc, j % 2 == 0, vslice[:, :, 0:64],
                  psv.rearrange("p (h e) -> p h e", e=64))
            nc.vector.tensor_copy(vslice[:, :, 64], ones8)


def _emit_qk_proj(nc, psp, wp, tp, qs, Wt, p, xtall, cos_sb, sin_sb,
                  dst_x1, dst_x2):
    """One p-group of a q/k projection + rope; writes DRAM tiles
    dst_x1/dst_x2 [128, S]."""
    w1all = wp.tile([128, NK, 128], F32R, tag="w", name="w1all")
    nc.sync.dma_start(
        out=w1all,
        in_=Wt[:, p * 128:(p + 1) * 128].rearrange("(k r) c -> r k c", r=128))
    w2all = wp.tile([128, NK, 128], F32R, tag="w", name="w2all")
    nc.sync.dma_start(
        out=w2all,
        in_=Wt[:, 256 + p * 128:256 + (p + 1) * 128].rearrange(
            "(k r) c -> r k c", r=128))
    wt1 = [w1all[:, k, :] for k in range(NK)]
    wt2 = [w2all[:, k, :] for k in range(NK)]
    for c in range(NC_):
        cs = slice(c * 512, (c + 1) * 512)
        ps1 = psp.tile([128, 512], F32, tag="pab", name="ps1")
        for k in range(NK):
            nc.tensor.matmul(ps1, wt1[k], xtall[:, k, cs],
                             start=(k == 0), stop=(k == NK - 1))
        ps2 = psp.tile([128, 512], F32, tag="pab", name="ps2")
        for k in range(NK):
            nc.tensor.matmul(ps2, wt2[k], xtall[:, k, cs],
                             start=(k == 0), stop=(k == NK - 1))
        tA = tp.tile([128, 512], F32, tag="rt", name="tA")
        nc.vector.tensor_mul(tA, ps1, cos_sb[:, cs])
        tB = tp.tile([128, 512], F32, tag="rt", name="tB")
        nc.vector.tensor_mul(tB, ps2, sin_sb[:, cs])
        o1 = qs.tile([128, 512], F32R, tag="ro", name="o1")
        nc.vector.tensor_sub(o1, tA, tB)
        nc.gpsimd.dma_start(out=dst_x1[:, cs], in_=o1)
        tC = tp.tile([128, 512], F32, tag="rt", name="tC")
        nc.vector.tensor_mul(tC, ps1, sin_sb[:, cs])
        tD = tp.tile([128, 512], F32, tag="rt", name="tD")
        nc.vector.tensor_mul(tD, ps2, cos_sb[:, cs])
        o2 = qs.tile([128, 512], F32R, tag="ro", name="o2")
        nc.vector.tensor_add(o2, tC, tD)
        nc.gpsimd.dma_start(out=dst_x2[:, cs], in_=o2)


def _emit_head(nc, psp, hp, ptp, rp, h, q_x1, q_x2, k_x1, k_x2, vpall,
               yd, tri):
    """Causal attention for head h, in sq-halves of HF with double-buffered
    score PSUM."""
    r = h % 4
    qth = hp.tile([64, S], F32R, tag="qth")
    nc.sync.dma_start(out=qth[0:32, :], in_=q_x1[r * 32:(r + 1) * 32, :])
    nc.sync.dma_start(out=qth[32:64, :], in_=q_x2[r * 32:(r + 1) * 32, :])
    kth = hp.tile([64, S], F32R, tag="kth")
    nc.sync.dma_start(out=kth[0:32, :], in_=k_x1[r * 32:(r + 1) * 32, :])
    nc.sync.dma_start(out=kth[32:64, :], in_=k_x2[r * 32:(r + 1) * 32, :])

    for half in range(2):
        base = half * HF
        pOt = psp.tile([128, HF], F32, tag="pO", bufs=1)
        pO = pOt[0:65, :]
        jmax_half = (base + HF - 1) // 128
        for j in range(jmax_half + 1):
            off = j * 128
            lo = max(off, base)
            hi = base + HF
            pS = psp.tile([128, HF], F32, tag="pS", bufs=2)
            s0 = lo
            while s0 < hi:
                s1 = min((s0 // 512 + 1) * 512, hi)
                nc.tensor.matmul(
                    pS[:, s0 - base:s1 - base],
                    kth[:, off:off + 128], qth[:, s0:s1],
                    start=True, stop=True)
                s0 = s1
            pt = ptp.tile([128, HF], F32R, tag="pt")
            nc.scalar.activation(pt[:, 0:hi - lo], pS[:, lo - base:HF],
                                 AF.Exp, scale=0.125)
            if lo == off:
                nc.vector.tensor_mul(pt[:, 0:128], pt[:, 0:128], tri)
            vsl = vpall[:, j, h * 65:(h + 1) * 65]
            for c in range(lo // 512, (base + HF) // 512):
                ss = max(c * 512, lo)
                se = (c + 1) * 512
                jmax = min(NS - 1, (se - 1) // 128)
                nc.tensor.matmul(
                    pO[:, ss - base:se - base], vsl, pt[:, ss - lo:se - lo],
                    start=(j == 0), stop=(j == jmax))

        oc_sb = rp.tile([65, HF], F32, tag="ocs", bufs=1)
        nc.vector.tensor_copy(oc_sb, pO)
        recip = rp.tile([1, HF], F32, tag="recip", bufs=1)
        nc.vector.reciprocal(recip, oc_sb[64:65, :])
        recb = rp.tile([64, HF], F32, tag="recb", bufs=1)
        nc.gpsimd.partition_broadcast(recb, recip)
        ys = rp.tile([64, HF], F32R, tag="ys", bufs=1)
        nc.vector.tensor_mul(ys, oc_sb[0:64, :], recb)
        nc.gpsimd.dma_start(out=yd[h * 64:(h + 1) * 64, base:base + HF],
                            in_=ys)


def _emit_oproj(nc, tc, psp, yd, OC, OT):
    """Partial output projection: OT = OC @ yT (y streamed from DRAM)."""
    with tc.tile_pool(name="op", bufs=4) as op, \
         tc.tile_pool(name="ost", bufs=4) as ost, \
         tc.tile_pool(name="yip", bufs=8) as yip:
        oct_sb = []
        for p in range(4):
            o_t = op.tile([128, D], F32R, tag="oct")
            nc.sync.dma_start(out=o_t, in_=OC[p * 128:(p + 1) * 128, :])
            oct_sb.append(o_t)
        for c in range(NC_):
            cs = slice(c * 512, (c + 1) * 512)
            yin = []
            for p in range(4):
                y_t = yip.tile([128, 512], F32R, tag="yin")
                nc.sync.dma_start(out=y_t, in_=yd[p * 128:(p + 1) * 128, cs])
                yin.append(y_t)
            for dt in range(8):
                pd = psp.tile([128, 512], F32, tag="pab", name="pd")
                for p in range(4):
                    nc.tensor.matmul(
                        pd, oct_sb[p][:, dt * 128:(dt + 1) * 128], yin[p],
                        start=(p == 0), stop=(p == 3))
                o_s = ost.tile([128, 512], F32, tag="os")
                _copy(nc, (dt + c) % 2 == 0, o_s, pd)
                nc.gpsimd.dma_start(
                    out=OT[dt * 128:(dt + 1) * 128, cs], in_=o_s)


def _build_nc():
    nc = bacc.Bacc(None, target_bir_lowering=False)

    XT = nc.dram_tensor("XT", [D, S], F32R, kind="ExternalInput")
    WQ = nc.dram_tensor("WQ", [D, HD], F32R, kind="ExternalInput")
    WK = nc.dram_tensor("WK", [D, HD], F32R, kind="ExternalInput")
    WV = nc.dram_tensor("WV", [D, HD], F32R, kind="ExternalInput")
    OC = nc.dram_tensor("OC", [HD, D], F32R, kind="ExternalInput")
    COS = nc.dram_tensor("COS", [128, S], F32, kind="ExternalInput")
    SIN = nc.dram_tensor("SIN", [128, S], F32, kind="ExternalInput")
    OT = nc.dram_tensor("OT", [D, S], F32, kind="ExternalOutput")

    with tile.TileContext(nc) as tc, ExitStack() as ctx:
        const = ctx.enter_context(tc.tile_pool(name="const", bufs=1))
        dram = ctx.enter_context(tc.tile_pool(name="dram", bufs=1,
                                              space="DRAM"))
        resv = ctx.enter_context(tc.tile_pool(name="resv", bufs=1))
        psp = ctx.enter_context(tc.tile_pool(name="psp", bufs=2,
                                             space="PSUM"))

        ident = None
        tri = const.tile([128, 128], F32, tag="tri")
        make_upper_triangular(nc, tri, val=1.0, diag=True)
        cos_sb = const.tile([128, S], F32, tag="cos")
        nc.sync.dma_start(out=cos_sb, in_=COS[:, :])
        sin_sb = const.tile([128, S], F32, tag="sin")
        nc.sync.dma_start(out=sin_sb, in_=SIN[:, :])
        ones8 = const.tile([128, 8], F32, tag="ones8")
        nc.vector.memset(ones8, 1.0)

        # per-(tensor, part, p) DRAM staging tiles for precise deps
        qk_dram = {}
        for tens in ("q", "k"):
            for part in ("x1", "x2"):
                for p in range(2):
                    qk_dram[(tens, part, p)] = dram.tile(
                        [128, S], F32R, tag=f"{tens}{part}{p}",
                        name=f"{tens}{part}{p}")

        vpall = resv.tile([128, NS, HPC * 65], F32R, tag="vpall")
        xtall = resv.tile([128, NK, S], F32R, tag="xtall")
        yd = dram.tile([HD, S], F32R, tag="yd")

        _emit_xT(nc, tc, psp, XT, xtall, ident)

        with tc.tile_pool(name="wp", bufs=4) as wp, \
             tc.tile_pool(name="tp", bufs=4) as tp, \
             tc.tile_pool(name="qs", bufs=2) as qs:
            _emit_qk_proj(nc, psp, wp, tp, qs, WQ, 0, xtall,
                          cos_sb, sin_sb,
                          qk_dram[("q", "x1", 0)], qk_dram[("q", "x2", 0)])
            _emit_qk_proj(nc, psp, wp, tp, qs, WK, 0, xtall,
                          cos_sb, sin_sb,
                          qk_dram[("k", "x1", 0)], qk_dram[("k", "x2", 0)])
            _emit_v(nc, tc, psp, WV, xtall, vpall, ones8)
            with tc.tile_pool(name="hp", bufs=2) as hp, \
                 tc.tile_pool(name="ptp", bufs=3) as ptp, \
                 tc.tile_pool(name="rp", bufs=2) as rp:
                for p in range(2):
                    if p == 1:
                        _emit_qk_proj(nc, psp, wp, tp, qs, WQ, 1, xtall,
                                      cos_sb, sin_sb,
                                      qk_dram[("q", "x1", 1)],
                                      qk_dram[("q", "x2", 1)])
                        _emit_qk_proj(nc, psp, wp, tp, qs, WK, 1, xtall,
                                      cos_sb, sin_sb,
                                      qk_dram[("k", "x1", 1)],
                                      qk_dram[("k", "x2", 1)])
                    for h in range(4 * p, 4 * p + 4):
                        _emit_head(nc, psp, hp, ptp, rp, h,
                                   qk_dram[("q", "x1", p)],
                                   qk_dram[("q", "x2", p)],
                                   qk_dram[("k", "x1", p)],
                                   qk_dram[("k", "x2", p)],
                                   vpall, yd, tri)

        _emit_oproj(nc, tc, psp, yd, OC, OT)

    nc.finalize()
    return nc


def _prep_inputs(x, q_proj, k_proj, v_proj, o_proj):
    pos = np.arange(S, dtype=np.float64)
    inv = THETA ** (-np.arange(0, DK, 2, dtype=np.float64) / DK)   # [32]
    ang = inv[:, None] * pos[None, :]                              # [32, S]
    cos_big = np.tile(np.cos(ang), (4, 1)).astype(np.float32)
    sin_big = np.tile(np.sin(ang), (4, 1)).astype(np.float32)

    in_maps = []
    for core in range(NCORES):
        b, g = core // 2, core % 2
        heads = [g * HPC + i for i in range(HPC)]
        rows_x1 = [h * DK + 2 * e for h in heads for e in range(32)]
        rows_x2 = [h * DK + 2 * e + 1 for h in heads for e in range(32)]
        perm = rows_x1 + rows_x2
        nat = [h * DK + d_ for h in heads for d_ in range(DK)]
        in_maps.append({
            "XT": np.ascontiguousarray(x[b].T, dtype=np.float32),
            "WQ": np.ascontiguousarray(q_proj[perm, :].T, dtype=np.float32),
            "WK": np.ascontiguousarray(k_proj[perm, :].T, dtype=np.float32),
            "WV": np.ascontiguousarray(v_proj[nat, :].T, dtype=np.float32),
            "OC": np.ascontiguousarray(o_proj[:, nat].T, dtype=np.float32),
            "COS": cos_big,
            "SIN": sin_big,
        })
    return in_maps


def _run(in_maps, **kw):
    if "nc" not in _CACHE:
        _CACHE["nc"] = _build_nc()
    return run_bass_kernel_spmd(_CACHE["nc"], in_maps,
                                core_ids=list(range(NCORES)), **kw)


def kernel(x, q_proj, k_proj, v_proj, o_proj):
    x = np.asarray(x, dtype=np.float32)
    in_maps = _prep_inputs(x,
                           np.asarray(q_proj, dtype=np.float32),
                           np.asarray(k_proj, dtype=np.float32),
                           np.asarray(v_proj, dtype=np.float32),
                           np.asarray(o_proj, dtype=np.float32))
    res = _run(in_maps)
    B = x.shape[0]
    out = np.empty((B, S, D), dtype=np.float32)
    for b in range(B):
        ot = res.results[2 * b]["OT"] + res.results[2 * b + 1]["OT"]
        out[b] = ot.T
    return out

